# revision 1
# baseline (speedup 1.0000x reference)
"""DigitCaps dynamic-routing kernel for Trainium2, 8 NeuronCores (SPMD).

Problem:  in_caps [64, 2048, 16] f32, W [2048, 32, 32, 16] f32
          u_hat[b,r,j,o] = sum_i W[r,j,o,i] * in_caps[b,r,i]
          3 routing iterations:
            c = softmax_j(b_ij);  s[b,j,o] = sum_r c[r,j] u_hat[b,r,j,o]
            v = squash_o(s);      b_ij += (1/BS) sum_{b,o} u_hat v
          returns v[..., None]  -> [64, 32, 32, 1]

Strategy (per core, routes sharded 256/core; K = (r,i) = 4096 rows):
  * W shard resident in SBUF as bf16 Wt[(r,i), (j,o)]; u_hat never
    materialized.  Each iteration:
      pass 1: s = (c-scaled Wt) contracted with uT on PE (K=4096, 32 chunks).
              One AllReduce of partial s [64, 1024] per iteration.
      pass 2: G = un.T @ v (PE);  b_upd = (1/64) sum_{i,o} Wt.G via
              DVE mult + o-reduce + i-reduce through a constant selector
              matmul accumulated in a persistent PSUM b_ij.
  * softmax / c-scale replicated over the 16 i-rows per route; the c scale
    is ACT-expanded over o to keep the DVE multiplies in 2x bf16 mode.
  * pass 2 of iteration t emits c chunk-by-chunk so pass 1 of t+1 overlaps.
"""

import numpy as np
import ml_dtypes

import concourse.bacc as bacc
import concourse.mybir as mybir
import concourse.tile as tile
from concourse.bass_utils import run_bass_kernel_spmd

BS, R, J, I, O = 64, 2048, 32, 16, 32
NUM_IT = 3
N_CORES = 8
R_LOC = R // N_CORES            # 256 routes per core
K_LOC = R_LOC * I               # 4096 contraction rows per core
NCHUNK = K_LOC // 128           # 32 chunks (8 routes x 16 i each)
JO = J * O                      # 1024
F32 = mybir.dt.float32
BF16 = mybir.dt.bfloat16
FP16 = mybir.dt.float16
AX = mybir.AxisListType
ALU = mybir.AluOpType
ACTF = mybir.ActivationFunctionType

WC_ON_GPSIMD = lambda b: b < 5          # 4-chunk wc batches: 5 gps-direct, 3 ACT-expand+DVE
TREE_ON_GPSIMD = lambda b: False        # o-reduce trees stay on vector
MULT_ON_GPSIMD = lambda b: False        # all W.G multiplies on vector


def _build_nc():
    nc = bacc.Bacc(trn_type="TRN2", target_bir_lowering=False, debug=False,
                   num_devices=N_CORES)
    wt = nc.dram_tensor("wt", [K_LOC, JO], FP16, kind="ExternalInput")
    ut = nc.dram_tensor("ut", [K_LOC, BS], FP16, kind="ExternalInput")
    un = nc.dram_tensor("un", [BS, K_LOC], FP16, kind="ExternalInput")
    sel = nc.dram_tensor("sel", [128, 128], FP16, kind="ExternalInput")
    vout = nc.dram_tensor("vout", [BS, JO], F32, kind="ExternalOutput")
    cc_wi = nc.dram_tensor("cc_wi", [1, 128], F32)
    cc_wo = nc.dram_tensor("cc_wo", [1, 128], F32, addr_space="Shared")
    cc_in = [nc.dram_tensor(f"cc_in{i}", [BS, JO], FP16) for i in range(NUM_IT)]
    cc_out = [nc.dram_tensor(f"cc_out{i}", [BS, JO], FP16, addr_space="Shared")
              for i in range(NUM_IT)]
    rg = [list(range(N_CORES))]

    with tile.TileContext(nc) as tc:
        with (
            tc.tile_pool(name="big", bufs=1) as big,
            tc.tile_pool(name="wc", bufs=3) as wcp,
            tc.tile_pool(name="cx", bufs=2) as cxp,
            tc.tile_pool(name="tmp", bufs=2) as tmpp,
            tc.tile_pool(name="gsb", bufs=3) as gsbp,
            tc.tile_pool(name="small", bufs=1) as small,
            tc.tile_pool(name="ps", bufs=3, space="PSUM") as psp,
            tc.tile_pool(name="bpsum", bufs=1, space="PSUM") as bpsum,
        ):
            # ---- resident tensors ----
            w_sb = big.tile([128, NCHUNK, JO], FP16)      # 64KB/part
            ut_sb = big.tile([128, NCHUNK, BS], FP16)
            un_sb = big.tile([BS, K_LOC], FP16)
            sel_sb = big.tile([128, 128], FP16)            # selector (1/64)
            e_rep = big.tile([128, NCHUNK, J], F32)       # exp(b) scratch
            c_rep = big.tile([128, NCHUNK, J], F32)       # c_ij replicated
            b_acc = bpsum.tile([128, NCHUNK, J], F32)     # persistent b_ij

            wt_v = wt.ap().rearrange("(c p) f -> c p f", p=128)
            ut_v = ut.ap().rearrange("(c p) f -> c p f", p=128)
            _dengs = [nc.sync, nc.scalar, nc.gpsimd]
            for c in range(NCHUNK):
                _dengs[c % 3].dma_start(out=w_sb[:, c, :], in_=wt_v[c])
            for c in range(NCHUNK):
                _dengs[c % 3].dma_start(out=ut_sb[:, c, :], in_=ut_v[c])
            nc.sync.dma_start(out=un_sb, in_=un.ap())
            nc.sync.dma_start(out=sel_sb, in_=sel.ap())
            # warm up the collective machinery under the weight load
            nc.gpsimd.collective_compute(
                "AllReduce", ALU.add, replica_groups=rg,
                ins=[cc_wi.ap()], outs=[cc_wo.ap()],
            )

            def emit_pass1(it):
                """c-scale + s-matmul accumulation for iteration `it`."""
                s_full = psp.tile([128, JO], F32, tag="ps")
                s_ps = s_full[:BS, :]
                for b in range(NCHUNK // 4):
                    c0 = 4 * b
                    if it == 0:
                        rhs_src = w_sb[:, c0:c0 + 4, :]
                    elif WC_ON_GPSIMD(b):
                        wc_t = wcp.tile([128, 4, JO], FP16)
                        for hb in range(2):
                            cs = c0 + 2 * hb
                            nc.gpsimd.tensor_tensor(
                                out=wc_t[:, 2 * hb:2 * hb + 2, :].rearrange(
                                    "p c (j o) -> p c j o", o=O),
                                in0=w_sb[:, cs:cs + 2, :].rearrange(
                                    "p c (j o) -> p c j o", o=O),
                                in1=c_rep[:, cs:cs + 2, :].unsqueeze(3)
                                    .broadcast_to([128, 2, J, O]),
                                op=ALU.mult)
                        rhs_src = wc_t
                    else:
                        c_exp = cxp.tile([128, 4, JO], FP16)
                        nc.scalar.copy(
                            c_exp.rearrange("p c (j o) -> p c j o", o=O),
                            c_rep[:, c0:c0 + 4, :].unsqueeze(3)
                                .broadcast_to([128, 4, J, O]))
                        wc_t = wcp.tile([128, 4, JO], FP16)
                        nc.vector.tensor_tensor(
                            out=wc_t, in0=w_sb[:, c0:c0 + 4, :], in1=c_exp,
                            op=ALU.mult)
                        rhs_src = wc_t
                    for ci in range(4):
                        for h in range(2):
                            nc.tensor.matmul(
                                out=s_ps[:, h * 512:(h + 1) * 512],
                                lhsT=ut_sb[:, c0 + ci, :],
                                rhs=rhs_src[:, ci, h * 512:(h + 1) * 512],
                                start=(c0 + ci == 0),
                                stop=(c0 + ci == NCHUNK - 1))
                return s_ps

            def emit_ar_squash(it, s_ps):
                """psum->AR->squash; returns v_sb [BS, J, O] f32."""
                s_sb = small.tile([BS, JO], FP16, tag="s_sb")
                if it == 0:
                    nc.scalar.mul(s_sb, s_ps, 1.0 / J)
                else:
                    nc.scalar.copy(s_sb, s_ps)
                for q in range(4):
                    deng = nc.sync if q % 2 == 0 else nc.scalar
                    deng.dma_start(
                        out=cc_in[it].ap()[:, q * 256:(q + 1) * 256],
                        in_=s_sb[:, q * 256:(q + 1) * 256])
                nc.gpsimd.collective_compute(
                    "AllReduce", ALU.add, replica_groups=rg,
                    ins=[cc_in[it].ap()], outs=[cc_out[it].ap()])
                s2 = small.tile([BS, J, O], FP16, tag=f"s2_{it % 2}")
                s2f = s2.rearrange("p j o -> p (j o)")
                for q in range(4):
                    deng = nc.sync if q % 2 == 0 else nc.scalar
                    deng.dma_start(
                        out=s2f[:, q * 256:(q + 1) * 256],
                        in_=cc_out[it].ap()[:, q * 256:(q + 1) * 256])
                ss = small.tile([BS, J, O], F32, tag="s_sb")
                nc.scalar.square(ss, s2)
                sq = small.tile([BS, J], F32)
                nc.vector.tensor_reduce(out=sq, in_=ss, axis=AX.X, op=ALU.add)
                rt = small.tile([BS, J], F32)
                nc.scalar.activation(rt, sq, ACTF.Sqrt)
                op1 = small.tile([BS, J], F32)
                nc.vector.tensor_scalar_add(op1, sq, 1.0)
                rden = small.tile([BS, J], F32)
                nc.vector.reciprocal(rden, op1)
                fac = small.tile([BS, J], F32)
                nc.vector.tensor_tensor(out=fac, in0=rt, in1=rden,
                                        op=ALU.mult)
                v_sb = small.tile([BS, J, O], F32, tag=f"v_{it % 2}")
                nc.vector.tensor_tensor(
                    out=v_sb, in0=s2,
                    in1=fac.unsqueeze(2).broadcast_to([BS, J, O]),
                    op=ALU.mult)
                return v_sb

            def emit_pass2(it, v_sb):
                """b_ij update + per-batch softmax refresh of c_rep."""
                v_r = small.tile([BS, JO], FP16, tag=f"vr{it % 2}")
                nc.scalar.copy(v_r, v_sb.rearrange("p j o -> p (j o)"))
                NB = NCHUNK // 4
                for b in range(NB):
                    c0 = 4 * b
                    g_sb = gsbp.tile([128, 4, JO], FP16)
                    for ci in range(4):
                        g_ps = psp.tile([128, JO], F32, tag="ps")
                        for h in range(2):
                            nc.tensor.matmul(
                                out=g_ps[:, h * 512:(h + 1) * 512],
                                lhsT=un_sb[:, (c0 + ci) * 128:
                                           (c0 + ci + 1) * 128],
                                rhs=v_r[:, h * 512:(h + 1) * 512],
                                start=True, stop=True)
                        nc.scalar.copy(g_sb[:, ci, :], g_ps)
                    tmp = tmpp.tile([128, 4, JO], FP16)
                    meng = nc.gpsimd if MULT_ON_GPSIMD(b) else nc.vector
                    meng.tensor_tensor(out=tmp, in0=w_sb[:, c0:c0 + 4, :],
                                       in1=g_sb, op=ALU.mult)
                    # o-reduction: two pairwise stages then selector matmuls
                    teng = nc.gpsimd if TREE_ON_GPSIMD(b) else nc.vector
                    ta = tmpp.tile([128, 128, 16], FP16, tag="ta")
                    t0 = tmp.rearrange("p c (j o) -> p (c j) o", o=O)
                    teng.tensor_tensor(out=ta, in0=t0[:, :, 0:16],
                                       in1=t0[:, :, 16:32], op=ALU.add)
                    tb = tmpp.tile([128, 128, 8], FP16, tag="tb")
                    teng.tensor_tensor(out=tb, in0=ta[:, :, 0:8],
                                       in1=ta[:, :, 8:16], op=ALU.add)
                    tcq = tmpp.tile([128, 128, 4], FP16, tag="tc")
                    teng.tensor_tensor(out=tcq, in0=tb[:, :, 0:4],
                                       in1=tb[:, :, 4:8], op=ALU.add)
                    td = tmpp.tile([128, 128, 2], FP16, tag="td")
                    teng.tensor_tensor(out=td, in0=tcq[:, :, 0:2],
                                       in1=tcq[:, :, 2:4], op=ALU.add)
                    for oo in range(2):
                        nc.tensor.matmul(
                            out=b_acc[:, c0:c0 + 4, :], lhsT=sel_sb,
                            rhs=td[:, :, oo],
                            start=(it == 0 and c0 % 16 == 0 and oo == 0),
                            stop=(it == NUM_IT - 2 and c0 % 16 == 12
                                  and oo == 1),
                            skip_group_check=True)
                    # softmax refresh for this batch (4 chunks)
                    nc.scalar.activation(e_rep[:, c0:c0 + 4, :],
                                         b_acc[:, c0:c0 + 4, :], ACTF.Exp)
                    esum = tmpp.tile([128, 4], F32, tag="esum")
                    nc.vector.tensor_reduce(
                        out=esum, in_=e_rep[:, c0:c0 + 4, :],
                        axis=AX.X, op=ALU.add)
                    erec = tmpp.tile([128, 4], F32, tag="erec")
                    nc.vector.reciprocal(erec, esum)
                    for cc in range(4):
                        nc.scalar.mul(c_rep[:, c0 + cc, :],
                                      e_rep[:, c0 + cc, :],
                                      erec[:, cc:cc + 1])

            v_sb = None
            for it in range(NUM_IT):
                s_ps = emit_pass1(it)
                v_sb = emit_ar_squash(it, s_ps)
                if it < NUM_IT - 1:
                    emit_pass2(it, v_sb)

            v_flat_out = v_sb.rearrange("p j o -> p (j o)")
            for q in range(4):
                nc.sync.dma_start(out=vout.ap()[:, q * 256:(q + 1) * 256],
                                  in_=v_flat_out[:, q * 256:(q + 1) * 256])
    nc.finalize()
    return nc


_NC_CACHE = {}
TRACE = False
TRACE_CORES = None


def _get_nc():
    if "nc" not in _NC_CACHE:
        _NC_CACHE["nc"] = _build_nc()
    return _NC_CACHE["nc"]


def _make_sel():
    sel = np.zeros((128, 128), np.float32)
    for p in range(128):
        m0 = (p // 16) * 16
        sel[p, m0:m0 + 16] = 1.0 / BS
    return sel


def kernel(**inputs):
    in_caps = np.ascontiguousarray(inputs["in_caps"], dtype=np.float32)
    W = np.ascontiguousarray(inputs["W"], dtype=np.float32)
    assert in_caps.shape == (BS, R, I) and W.shape == (R, J, O, I)

    bf = np.float16
    Wt = np.ascontiguousarray(
        W.transpose(0, 3, 1, 2).reshape(R * I, J * O).astype(bf))
    uT = np.ascontiguousarray(
        in_caps.transpose(1, 2, 0).reshape(R * I, BS).astype(bf))
    un = np.ascontiguousarray(in_caps.reshape(BS, R * I).astype(bf))
    sel = _make_sel().astype(np.float16)

    in_maps = []
    for k in range(N_CORES):
        rows = slice(k * K_LOC, (k + 1) * K_LOC)
        in_maps.append({
            "wt": np.ascontiguousarray(Wt[rows]),
            "ut": np.ascontiguousarray(uT[rows]),
            "un": np.ascontiguousarray(un[:, rows]),
            "sel": sel,
        })

    nc = _get_nc()
    res = run_bass_kernel_spmd(nc, in_maps, core_ids=list(range(N_CORES)),
                               trace=TRACE, trace_cores=TRACE_CORES)
    _NC_CACHE["last_result"] = res
    v = np.asarray(res.results[0]["vout"], dtype=np.float32)
    return v.reshape(BS, J, O, 1)


if __name__ == "__main__":
    rng = np.random.default_rng(0)
    ins = {
        "in_caps": rng.standard_normal((BS, R, I), dtype=np.float32),
        "W": rng.standard_normal((R, J, O, I), dtype=np.float32),
    }
    out = kernel(**ins)
    print(out.shape, out.dtype, np.abs(out).mean())



# revision 6
# speedup vs baseline: 1.0127x; 1.0127x over previous
"""DigitCaps dynamic-routing kernel for Trainium2, 8 NeuronCores (SPMD).

Problem:  in_caps [64, 2048, 16] f32, W [2048, 32, 32, 16] f32
          u_hat[b,r,j,o] = sum_i W[r,j,o,i] * in_caps[b,r,i]
          3 routing iterations:
            c = softmax_j(b_ij);  s[b,j,o] = sum_r c[r,j] u_hat[b,r,j,o]
            v = squash_o(s);      b_ij += (1/BS) sum_{b,o} u_hat v
          returns v[..., None]  -> [64, 32, 32, 1]

Strategy (per core, routes sharded 256/core; K = (r,i) = 4096 rows):
  * W shard resident in SBUF as fp16 Wt[(r,i), (j,o)]; u_hat never
    materialized.  Each iteration:
      pass 1: s = (c-scaled Wt) contracted with uT on PE (K=4096).
              One AllReduce of partial s [64, 1024] per iteration.
      pass 2: G = un.T @ v (PE);  tmp = Wt*G on DVE (2x mode);
              o-reduce via one segmented tensor_reduce; i-reduce via a
              constant selector matmul accumulated in a persistent PSUM
              b_ij.  Softmax refreshed per 2-chunk group so pass 1 of
              the next iteration pipelines behind pass 2.
  * c is stored in duplicated-pair layout c2[p, chunk, j, 2] so the
    broadcast over o keeps the DVE multiply in 2x fp16 mode with no
    ACT-side expansion pass.
"""

import numpy as np

import concourse.bacc as bacc
import concourse.mybir as mybir
import concourse.tile as tile
from concourse.bass_utils import run_bass_kernel_spmd

BS, R, J, I, O = 64, 2048, 32, 16, 32
NUM_IT = 3
N_CORES = 8
R_LOC = R // N_CORES            # 256 routes per core
K_LOC = R_LOC * I               # 4096 contraction rows per core
NCHUNK = K_LOC // 128           # 32 chunks (8 routes x 16 i each)
GRP = 2                         # chunks per pipeline group
NGRP = NCHUNK // GRP            # 16 groups
JO = J * O                      # 1024
F32 = mybir.dt.float32
FP16 = mybir.dt.float16
AX = mybir.AxisListType
ALU = mybir.AluOpType
ACTF = mybir.ActivationFunctionType


def _build_nc():
    nc = bacc.Bacc(trn_type="TRN2", target_bir_lowering=False, debug=False,
                   num_devices=N_CORES)
    wt = nc.dram_tensor("wt", [K_LOC, JO], FP16, kind="ExternalInput")
    ut = nc.dram_tensor("ut", [K_LOC, BS], FP16, kind="ExternalInput")
    un = nc.dram_tensor("un", [BS, K_LOC], FP16, kind="ExternalInput")
    sel = nc.dram_tensor("sel", [128, 128], FP16, kind="ExternalInput")
    vout = nc.dram_tensor("vout", [BS, JO], F32, kind="ExternalOutput")
    cc_wi = nc.dram_tensor("cc_wi", [BS, JO], FP16)
    cc_wo = nc.dram_tensor("cc_wo", [BS, JO], FP16, addr_space="Shared")
    cc_in = [nc.dram_tensor(f"cc_in{i}", [BS, JO], FP16) for i in range(NUM_IT)]
    cc_out = [nc.dram_tensor(f"cc_out{i}", [BS, JO], FP16, addr_space="Shared")
              for i in range(NUM_IT)]
    rg = [list(range(N_CORES))]

    with tile.TileContext(nc) as tc:
        with (
            tc.tile_pool(name="big", bufs=1) as big,
            tc.tile_pool(name="wc", bufs=3) as wcp,
            tc.tile_pool(name="gsb", bufs=3) as gsbp,
            tc.tile_pool(name="wg", bufs=3) as wgp,
            tc.tile_pool(name="sm", bufs=2) as smp,
            tc.tile_pool(name="small", bufs=1) as small,
            tc.tile_pool(name="sps", bufs=1, space="PSUM") as spsp,
            tc.tile_pool(name="gps", bufs=2, space="PSUM") as gpsp,
            tc.tile_pool(name="bpsum", bufs=1, space="PSUM") as bpsum,
        ):
            # ---- resident tensors ----
            w_sb = big.tile([128, NCHUNK, JO], FP16)      # 64KB/part
            ut_sb = big.tile([128, NCHUNK, BS], FP16)
            un_sb = big.tile([BS, K_LOC], FP16)
            sel_sb = big.tile([128, 128], FP16)            # selector (1/64)
            e_sb = big.tile([128, NCHUNK, J], F32)        # exp(b) scratch
            c2_sb = big.tile([128, NCHUNK, J, 2], FP16)   # c_ij dup-pairs
            b_acc = bpsum.tile([128, NCHUNK, J], F32)     # persistent b_ij

            wt_v = wt.ap().rearrange("(c p) f -> c p f", p=128)
            ut_v = ut.ap().rearrange("(c p) f -> c p f", p=128)
            # warm up the collective machinery first (gpsimd queue)
            nc.gpsimd.collective_compute(
                "AllReduce", ALU.add, replica_groups=rg,
                ins=[cc_wi.ap()], outs=[cc_wo.ap()],
            )
            nc.sync.dma_start(out=un_sb, in_=un.ap())
            nc.sync.dma_start(out=sel_sb, in_=sel.ap())
            # interleave ut+w chunk loads over the 3 DMA queues so pass1
            # it0 can start as soon as the first chunks land
            _dengs = [nc.sync, nc.scalar, nc.gpsimd]
            for c in range(NCHUNK):
                eng = _dengs[c % 3]
                eng.dma_start(out=ut_sb[:, c, :], in_=ut_v[c])
                eng.dma_start(out=w_sb[:, c, :], in_=wt_v[c])

            state = {}

            def emit_pass1_group(it, g):
                if g == 0:
                    state["s_ps"] = spsp.tile([128, JO], F32, tag="s",
                                              name="s_ps")
                s_ps = state["s_ps"]
                c0 = GRP * g
                if it == 0:
                    rhs = w_sb[:, c0:c0 + GRP, :]
                else:
                    wc_t = wcp.tile([128, GRP, JO], FP16)
                    nc.vector.tensor_tensor(
                        out=wc_t.rearrange("p c (j o2 t) -> p c j o2 t",
                                           o2=O // 2, t=2),
                        in0=w_sb[:, c0:c0 + GRP, :].rearrange(
                            "p c (j o2 t) -> p c j o2 t", o2=O // 2, t=2),
                        in1=c2_sb[:, c0:c0 + GRP, :, :].unsqueeze(3)
                            .broadcast_to([128, GRP, J, O // 2, 2]),
                        op=ALU.mult)
                    rhs = wc_t
                for ci in range(GRP):
                    for h in range(2):
                        nc.tensor.matmul(
                            out=s_ps[:BS, h * 512:(h + 1) * 512],
                            lhsT=ut_sb[:, c0 + ci, :],
                            rhs=rhs[:, ci, h * 512:(h + 1) * 512],
                            start=(c0 + ci == 0),
                            stop=(c0 + ci == NCHUNK - 1))

            def emit_ar_squash(it):
                """psum->AR->squash; returns (v_sb f32, v_r fp16)."""
                s_ps = state["s_ps"]
                s_sb = small.tile([BS, JO], FP16, tag="s_sb")
                if it == 0:
                    nc.scalar.mul(s_sb, s_ps[:BS], 1.0 / J)
                else:
                    nc.scalar.copy(s_sb, s_ps[:BS])
                for q in range(4):
                    deng = nc.sync if q % 2 == 0 else nc.scalar
                    deng.dma_start(
                        out=cc_in[it].ap()[:, q * 256:(q + 1) * 256],
                        in_=s_sb[:, q * 256:(q + 1) * 256])
                nc.gpsimd.collective_compute(
                    "AllReduce", ALU.add, replica_groups=rg,
                    ins=[cc_in[it].ap()], outs=[cc_out[it].ap()])
                s2 = small.tile([BS, J, O], FP16, tag=f"s2_{it % 2}")
                s2f = s2.rearrange("p j o -> p (j o)")
                for q in range(4):
                    deng = nc.sync if q % 2 == 0 else nc.scalar
                    deng.dma_start(
                        out=s2f[:, q * 256:(q + 1) * 256],
                        in_=cc_out[it].ap()[:, q * 256:(q + 1) * 256])
                ss = small.tile([BS, J, O], F32, tag="ss")
                nc.scalar.square(ss, s2)
                sq = small.tile([BS, J], F32)
                nc.vector.tensor_reduce(out=sq, in_=ss, axis=AX.X, op=ALU.add)
                rt = small.tile([BS, J], F32)
                nc.scalar.activation(rt, sq, ACTF.Sqrt)
                op1 = small.tile([BS, J], F32)
                nc.vector.tensor_scalar_add(op1, sq, 1.0)
                rden = small.tile([BS, J], F32)
                nc.vector.reciprocal(rden, op1)
                fac = small.tile([BS, J], F32)
                nc.vector.tensor_tensor(out=fac, in0=rt, in1=rden,
                                        op=ALU.mult)
                v_sb = small.tile([BS, J, O], F32, tag=f"v_{it % 2}")
                nc.vector.tensor_tensor(
                    out=v_sb, in0=s2,
                    in1=fac.unsqueeze(2).broadcast_to([BS, J, O]),
                    op=ALU.mult)
                v_r = small.tile([BS, JO], FP16, tag=f"vr{it % 2}")
                nc.scalar.copy(v_r, v_sb.rearrange("p j o -> p (j o)"))
                return v_sb, v_r

            def emit_pass2_group(it, g, v_r):
                """b_ij update + softmax refresh for one 2-chunk group."""
                c0 = GRP * g
                g_sb = gsbp.tile([128, GRP, JO], FP16)
                for ci in range(GRP):
                    g_ps = gpsp.tile([128, JO], F32, tag="g")
                    for h in range(2):
                        nc.tensor.matmul(
                            out=g_ps[:, h * 512:(h + 1) * 512],
                            lhsT=un_sb[:, (c0 + ci) * 128:
                                       (c0 + ci + 1) * 128],
                            rhs=v_r[:, h * 512:(h + 1) * 512],
                            start=True, stop=True)
                    nc.scalar.copy(g_sb[:, ci, :], g_ps)
                wg = wgp.tile([128, GRP, JO], FP16)
                nc.vector.tensor_tensor(out=wg, in0=w_sb[:, c0:c0 + GRP, :],
                                        in1=g_sb, op=ALU.mult)
                bup = smp.tile([128, GRP * J], FP16, tag="bup")
                with nc.allow_low_precision("bup fp16 is enough for logits"):
                    nc.vector.tensor_reduce(
                        out=bup,
                        in_=wg.rearrange("p c (j o) -> p (c j) o", o=O),
                        axis=AX.X, op=ALU.add)
                # b_acc spans 2 PSUM banks; start/stop once per bank
                nc.tensor.matmul(
                    out=b_acc[:, c0:c0 + GRP, :], lhsT=sel_sb, rhs=bup,
                    start=(it == 0 and g in (0, NGRP // 2)),
                    stop=(it == NUM_IT - 2 and g in (NGRP // 2 - 1, NGRP - 1)),
                    skip_group_check=True)
                # softmax refresh for this group
                nc.scalar.activation(e_sb[:, c0:c0 + GRP, :],
                                     b_acc[:, c0:c0 + GRP, :], ACTF.Exp)
                esum = smp.tile([128, GRP], F32, tag="esum")
                nc.vector.tensor_reduce(
                    out=esum, in_=e_sb[:, c0:c0 + GRP, :],
                    axis=AX.X, op=ALU.add)
                erec = smp.tile([128, GRP], F32, tag="erec")
                nc.vector.reciprocal(erec, esum)
                nc.vector.tensor_tensor(
                    out=c2_sb[:, c0:c0 + GRP, :, :],
                    in0=e_sb[:, c0:c0 + GRP, :].unsqueeze(3)
                        .broadcast_to([128, GRP, J, 2]),
                    in1=erec.unsqueeze(2).unsqueeze(3)
                        .broadcast_to([128, GRP, J, 2]),
                    op=ALU.mult)

            # ---- main schedule ----
            for g in range(NGRP):
                emit_pass1_group(0, g)
            v_sb = None
            for it in range(NUM_IT):
                v_sb, v_r = emit_ar_squash(it)
                if it < NUM_IT - 1:
                    for g in range(NGRP):
                        emit_pass2_group(it, g, v_r)
                        emit_pass1_group(it + 1, g)

            v_flat_out = v_sb.rearrange("p j o -> p (j o)")
            for q in range(4):
                nc.sync.dma_start(out=vout.ap()[:, q * 256:(q + 1) * 256],
                                  in_=v_flat_out[:, q * 256:(q + 1) * 256])
    nc.finalize()
    return nc


_NC_CACHE = {}
TRACE = False
TRACE_CORES = None


def _get_nc():
    if "nc" not in _NC_CACHE:
        _NC_CACHE["nc"] = _build_nc()
    return _NC_CACHE["nc"]


def _make_sel():
    sel = np.zeros((128, 128), np.float32)
    for p in range(128):
        m0 = (p // 16) * 16
        sel[p, m0:m0 + 16] = 1.0 / BS
    return sel


def kernel(**inputs):
    in_caps = np.ascontiguousarray(inputs["in_caps"], dtype=np.float32)
    W = np.ascontiguousarray(inputs["W"], dtype=np.float32)
    assert in_caps.shape == (BS, R, I) and W.shape == (R, J, O, I)

    bf = np.float16
    Wt = np.ascontiguousarray(
        W.transpose(0, 3, 1, 2).reshape(R * I, J * O).astype(bf))
    uT = np.ascontiguousarray(
        in_caps.transpose(1, 2, 0).reshape(R * I, BS).astype(bf))
    un = np.ascontiguousarray(in_caps.reshape(BS, R * I).astype(bf))
    sel = _make_sel().astype(np.float16)

    in_maps = []
    for k in range(N_CORES):
        rows = slice(k * K_LOC, (k + 1) * K_LOC)
        in_maps.append({
            "wt": np.ascontiguousarray(Wt[rows]),
            "ut": np.ascontiguousarray(uT[rows]),
            "un": np.ascontiguousarray(un[:, rows]),
            "sel": sel,
        })

    nc = _get_nc()
    res = run_bass_kernel_spmd(nc, in_maps, core_ids=list(range(N_CORES)),
                               trace=TRACE, trace_cores=TRACE_CORES)
    _NC_CACHE["last_result"] = res
    v = np.asarray(res.results[0]["vout"], dtype=np.float32)
    return v.reshape(BS, J, O, 1)


if __name__ == "__main__":
    rng = np.random.default_rng(0)
    ins = {
        "in_caps": rng.standard_normal((BS, R, I), dtype=np.float32),
        "W": rng.standard_normal((R, J, O, I), dtype=np.float32),
    }
    out = kernel(**ins)
    print(out.shape, out.dtype, np.abs(out).mean())
